# revision 1
# baseline (speedup 1.0000x reference)
"""Trainium2 Bass kernel for nn_Decoder (attention-LSTM decoder + vocab projection).

Sharding (8 NeuronCores, SPMD, rank-agnostic program; all rank dependence is
carried by per-core input data):
  - Hidden dim H=1024 (and matching i/f/g/o gate rows) sharded 8 ways.
  - Per decode step, two small AllGathers: AG1 carries [h2^T slice | partial
    attention scores]; AG2 carries the ctx^T slice. Everything else is local.
  - The output projection (dominant FLOPs) is vocab-sharded; the h2 history
    is accumulated in SBUF during the loop and projected in one pass at the
    end, streaming each core's Wout slice from HBM exactly once.

Precision: the LSTM recurrence amplifies per-step rounding noise by ~1000x
over 64 steps, so every matmul feeding the recurrence runs as an fp16 hi/lo
split (3 cross terms, fp32 PSUM accumulation => ~1e-6 error, measured on HW)
at full 1 cycle/row PE speed. The lhsT hi and lo parts are stacked along the
M (partition) axis so the 3 terms cost only 2 passes over the rhs. The final
vocab projection does not feed back, so it uses single-pass float32r
(~1.6e-4). Elementwise/state math is exact fp32.
"""

import sys

sys.path.insert(0, "/opt/trn_rl_repo")

import numpy as np

import concourse.mybir as mybir
import concourse.tile as tile
from concourse import bacc, bass_utils
from concourse.masks import make_identity

P = 128
B, TENC, V, E, H, A = 32, 128, 32000, 512, 1024, 128
NCORES = 8
HSL = H // NCORES          # 128 h-dims per core
GSL = 4 * HSL              # 512 gate rows per core
VSL = V // NCORES          # 4000 vocab per core
NT = 500                   # projection N chunk (4000 = 8 x 500)
KT = H // P                # 8 k-tiles over the hidden dim

f32 = mybir.dt.float32
f32r = mybir.dt.float32r
f16 = mybir.dt.float16
ADD = mybir.AluOpType.add
SUB = mybir.AluOpType.subtract
MUL = mybir.AluOpType.mult
AF = mybir.ActivationFunctionType

_CACHE = {}


def build_decoder(T):
    TB = T * B
    MT = TB // P
    nc = bacc.Bacc("TRN2", target_bir_lowering=False, debug=False,
                   num_devices=NCORES)

    def din(name, shape, dt_):
        return nc.dram_tensor(name, shape, dt_, kind="ExternalInput")

    # fp16 hi/lo pairs are prepared host-side for all static operands
    d_xeat = [din(f"xeat_{s}", [640, TB], f16) for s in "hl"]
    d_weat = [din(f"weat_{s}", [640, GSL], f16) for s in "hl"]
    d_wct = [din(f"wct_{s}", [H, GSL], f16) for s in "hl"]
    d_whht = [din(f"whht_{s}", [H, GSL], f16) for s in "hl"]
    d_watj = [din(f"watj_{s}", [H, HSL], f16) for s in "hl"]
    d_enctr = [din(f"enctr_{s}", [P, KT, B, TENC], f16) for s in "hl"]
    d_enctbj = [din(f"enctbj_{s}", [TENC, B * HSL], f16) for s in "hl"]
    d_woutt = din("woutt", [H, VSL], f16)
    d_h0tj = din("h0tj", [HSL, B], f32)
    d_biasg = din("biasg", [1, GSL], f32)
    d_maskb = din("maskb", [B, TENC], f32)
    d_c0j = din("c0j", [B, HSL], f32)
    d_logits = nc.dram_tensor("logits", [TB, VSL], f32, kind="ExternalOutput")

    rg = [list(range(NCORES))]

    with tile.TileContext(nc) as tc:
      with tc.tile_pool(name="const", bufs=1) as const, \
           tc.tile_pool(name="dramc", bufs=1, space="DRAM") as dramc, \
           tc.tile_pool(name="dram2", bufs=2, space="DRAM") as dram2, \
           tc.tile_pool(name="ps512", bufs=3, space="PSUM") as ps512, \
           tc.tile_pool(name="ps128", bufs=5, space="PSUM") as ps128, \
           tc.tile_pool(name="work", bufs=2) as work, \
           tc.tile_pool(name="wop", bufs=2) as wop, \
           tc.tile_pool(name="otp", bufs=2) as otp:

        def ctile(shape, dt_, name):
            return const.tile(shape, dt_, name=name, tag=name)

        ident = ctile([P, P], f32, "ident")
        make_identity(nc, ident[:])
        maskb_sb = ctile([B, TENC], f32, "maskb_sb")
        nc.sync.dma_start(maskb_sb[:], d_maskb.ap())

        # ---- persistent P2 operands (fp16 hi/lo pairs) ----
        encat = [ctile([P, B, TENC], f16, f"encat_{s}") for s in "hl"]
        enctbj = [ctile([P, B, HSL], f16, f"enctbj_{s}") for s in "hl"]
        whht_sb = [ctile([P, KT, GSL], f16, f"whht_{s}") for s in "hl"]
        wct_sb = [ctile([P, KT, GSL], f16, f"wct_{s}") for s in "hl"]
        c_st = ctile([B, HSL], f32, "c_st")
        hT = ctile([P, KT, B], f32, "hT")
        hTs = ctile([P, KT, 2 * B], f16, "hTs")      # [hi | lo] stacked on M
        ctxTs = ctile([P, KT, 2 * B], f16, "ctxTs")
        h2T_loc = ctile([HSL, B], f32, "h2T_loc")
        # per-b stacked block-diag lhsT tiles: cols [64b:64b+32] = hi diag,
        # [64b+32 : 64b+64] = lo diag (diag entry at col offset 65*b)
        scblk = ctile([P, 65 * B + B], f16, "scblk")
        atblk = ctile([P, 65 * B + B], f16, "atblk")
        nc.vector.memset(scblk[:], 0.0)
        nc.vector.memset(atblk[:], 0.0)
        h2tf = ctile([P, KT, TB], f16, "h2tf")  # all steps of h^T
        xea_dram = dramc.tile([P, MT, GSL], f32, name="xea_dram", tag="xea_dram")

        def diag(blk, off):
            # (128, 32) view with free stride 65: cols off + 65*b
            return blk[:, off:off + 65 * B].rearrange(
                "p (a c) -> p a c", c=65)[:, :, 0]

        for s in (0, 1):
            nc.sync.dma_start(
                enctbj[s][:],
                d_enctbj[s].ap().rearrange("t (b j) -> t b j", j=HSL))
            nc.sync.dma_start(
                whht_sb[s][:], d_whht[s].ap().rearrange("(kt p) g -> p kt g", p=P))
            nc.sync.dma_start(
                wct_sb[s][:], d_wct[s].ap().rearrange("(kt p) g -> p kt g", p=P))
        nc.sync.dma_start(c_st[:], d_c0j.ap())
        nc.sync.dma_start(h2T_loc[:], d_h0tj.ap())

        # ---------------- P1: Xea + EncA^T precomputes ----------------
        with tc.tile_pool(name="p1", bufs=2) as p1, \
             tc.tile_pool(name="p1c", bufs=1) as p1c:
            onesf = p1c.tile([1, P], f32)
            nc.vector.memset(onesf[:], 1.0)
            biasg_sb = p1c.tile([1, GSL], f32)
            nc.sync.dma_start(biasg_sb[:], d_biasg.ap())
            biasb = p1c.tile([P, GSL], f32)
            pb = ps512.tile([P, GSL], f32, name="pb", tag="ps512")
            nc.tensor.matmul(pb[:], onesf[:], biasg_sb[:], start=True, stop=True)
            nc.vector.tensor_copy(out=biasb[:], in_=pb[:])
            weat_sb = [p1c.tile([P, 5, GSL], f16, name=f"weat{s}") for s in "hl"]
            for s in (0, 1):
                nc.sync.dma_start(
                    weat_sb[s][:],
                    d_weat[s].ap().rearrange("(kt p) g -> p kt g", p=P))
            # Xea[(t,b), g] = [emb|add] @ Wea + bias   (3-term fp16 split)
            for mt in range(MT):
                xin = [p1.tile([P, 5, P], f16, tag=f"xin{s}", name=f"xin{s}")
                       for s in "hl"]
                for s in (0, 1):
                    nc.sync.dma_start(
                        xin[s][:],
                        d_xeat[s].ap().rearrange("(kt p) m -> p kt m", p=P)
                        [:, :, mt * P:(mt + 1) * P])
                px = ps512.tile([P, GSL], f32, name="px", tag="ps512")
                first = True
                for (a, w) in ((0, 0), (0, 1), (1, 0)):
                    for kt in range(5):
                        nc.tensor.matmul(px[:], xin[a][:, kt, :],
                                         weat_sb[w][:, kt, :],
                                         start=first, stop=(a == 1 and kt == 4))
                        first = False
                xsb = p1.tile([P, GSL], f32, tag="xsb", name="xsb")
                nc.vector.tensor_tensor(out=xsb[:], in0=px[:],
                                        in1=biasb[:], op=ADD)
                nc.sync.dma_start(xea_dram[:, mt, :], xsb[:])

            watj_sb = [p1c.tile([P, KT, HSL], f16, name=f"watj{s}") for s in "hl"]
            for s in (0, 1):
                nc.sync.dma_start(
                    watj_sb[s][:],
                    d_watj[s].ap().rearrange("(kt p) j -> p kt j", p=P))
            # EncA^T[j, b, t] = Wa[jsl, :] @ enc[b]^T  (3-term, evict hi/lo)
            for b in range(B):
                etr = [p1.tile([P, KT, TENC], f16, tag=f"etr{s}",
                               name=f"etr{s}") for s in "hl"]
                for s in (0, 1):
                    nc.sync.dma_start(
                        etr[s][:], d_enctr[s].ap()[:, :, b, :])
                pa = ps512.tile([P, TENC], f32, name="pa", tag="ps512")
                first = True
                for (w, a) in ((0, 0), (0, 1), (1, 0)):
                    for kt in range(KT):
                        nc.tensor.matmul(
                            pa[:], watj_sb[w][:, kt, :], etr[a][:, kt, :],
                            start=first,
                            stop=(w == 1 and a == 0 and kt == KT - 1))
                        first = False
                tmpa = p1.tile([P, TENC], f32, tag="tmpa", name="tmpa")
                nc.scalar.activation(encat[0][:, b, :], pa[:], AF.Copy)
                nc.vector.tensor_tensor(out=tmpa[:], in0=pa[:],
                                        in1=encat[0][:, b, :], op=SUB)
                nc.scalar.activation(encat[1][:, b, :], tmpa[:], AF.Copy)

        # ---------------- P2: recurrent loop ----------------
        for t in range(T + 1):
            last = t == T
            # ---- score partials from own h slice ----
            if not last:
                h2hi = work.tile([HSL, B], f16, tag="h2hi", name="h2hi")
                nc.scalar.activation(h2hi[:], h2T_loc[:], AF.Copy)
                h2lo = work.tile([HSL, B], f32, tag="h2lo", name="h2lo")
                nc.vector.tensor_tensor(out=h2lo[:], in0=h2T_loc[:],
                                        in1=h2hi[:], op=SUB)
                nc.vector.tensor_copy(out=diag(scblk, 0), in_=h2hi[:])
                nc.vector.tensor_copy(out=diag(scblk, B), in_=h2lo[:])
                ps_sc = ps128.tile([2 * B, TENC], f32, name="ps_sc", tag="ps128")
                first = True
                for w in (0, 1):
                    wid = 2 * B if w == 0 else B
                    for b in range(B):
                        nc.tensor.matmul(
                            ps_sc[0:wid, :], scblk[:, 2 * B * b:2 * B * b + wid],
                            encat[w][:, b, :],
                            start=first, stop=(w == 1 and b == B - 1))
                        first = False
                sc_lo = work.tile([B, TENC], f32, tag="sc_lo", name="sc_lo")
                nc.scalar.activation(sc_lo[:], ps_sc[B:2 * B, :], AF.Copy)
                sc_sb = work.tile([B, TENC], f32, tag="sc_sb", name="sc_sb")
                nc.vector.tensor_tensor(out=sc_sb[:], in0=ps_sc[0:B, :],
                                        in1=sc_lo[:], op=ADD)

            # ---- AG1: [h2T | score partial] ----
            pay = B * HSL
            bounce = dram2.tile([2 * pay], f32, name=f"bounce_{t}", tag="bounce")
            agout = dram2.tile([NCORES, 2 * pay], f32, addr_space="Shared",
                               name=f"agout_{t}", tag="agout")
            nc.sync.dma_start(
                bounce[0:pay].rearrange("(p f) -> p f", f=B), h2T_loc[:])
            if not last:
                nc.sync.dma_start(
                    bounce[pay:2 * pay].rearrange("(c f) -> c f", f=TENC),
                    sc_sb[:])
            nc.gpsimd.collective_compute(
                "AllGather", mybir.AluOpType.bypass, replica_groups=rg,
                ins=[bounce.opt()], outs=[agout.opt()])
            nc.sync.dma_start(
                hT[:], agout[:, 0:pay].rearrange("r (p f) -> p r f", f=B))

            # stash h^T (h2 of step t-1) for the end-of-loop projection
            if t >= 1:
                nc.scalar.activation(h2tf[:, :, B * (t - 1):B * t], hT[:],
                                     AF.Copy)
            if last:
                break

            # hi/lo stack of full h^T (for the Whh matmul)
            nc.scalar.activation(hTs[:, :, 0:B], hT[:], AF.Copy)
            tmph = work.tile([P, KT, B], f32, tag="tmph", name="tmph")
            nc.vector.tensor_tensor(out=tmph[:], in0=hT[:],
                                    in1=hTs[:, :, 0:B], op=SUB)
            nc.scalar.activation(hTs[:, :, B:2 * B], tmph[:], AF.Copy)

            # gates psum: h part first (independent of softmax)
            ps_g = ps512.tile([2 * B, GSL], f32, name="ps_g", tag="ps512")
            for kt in range(KT):
                nc.tensor.matmul(ps_g[:], hTs[:, kt, :], whht_sb[0][:, kt, :],
                                 start=(kt == 0), stop=False)
            for kt in range(KT):
                nc.tensor.matmul(ps_g[0:B, :], hTs[:, kt, 0:B],
                                 whht_sb[1][:, kt, :], start=False, stop=False)

            # ---- scores -> softmax ----
            sc8 = work.tile([B, NCORES, TENC], f32, tag="sc8", name="sc8", bufs=1)
            nc.sync.dma_start(
                sc8[:],
                agout[:, pay:2 * pay].rearrange("r (c f) -> c r f", f=TENC))
            scores = work.tile([B, TENC], f32, tag="scores", name="scores")
            nc.vector.reduce_sum(scores[:], sc8[:].rearrange("c r f -> c f r"),
                                 axis=mybir.AxisListType.X)
            nc.vector.tensor_tensor(out=scores[:], in0=scores[:],
                                    in1=maskb_sb[:], op=ADD)
            negmax = work.tile([B, 1], f32, tag="negmax", name="negmax")
            nc.vector.reduce_max(negmax[:], scores[:],
                                 axis=mybir.AxisListType.X, negate=True)
            attn_e = work.tile([B, TENC], f32, tag="attn_e", name="attn_e")
            sumexp = work.tile([B, 1], f32, tag="sumexp", name="sumexp")
            nc.scalar.activation(attn_e[:], scores[:], AF.Exp,
                                 bias=negmax[:], scale=1.0, accum_out=sumexp[:])
            recip = work.tile([B, 1], f32, tag="recip", name="recip")
            nc.vector.reciprocal(recip[:], sumexp[:])
            attn_n = work.tile([B, TENC], f32, tag="attn_n", name="attn_n")
            nc.vector.tensor_scalar_mul(attn_n[:], attn_e[:], recip[:])

            # attn^T hi/lo into block-diag
            ps_at = ps128.tile([TENC, B], f32, name="ps_at", tag="ps128")
            nc.tensor.transpose(ps_at[:], attn_n[:], ident[0:B, 0:B])
            athi = work.tile([TENC, B], f16, tag="athi", name="athi")
            nc.scalar.activation(athi[:], ps_at[:], AF.Copy)
            atlo = work.tile([TENC, B], f32, tag="atlo", name="atlo")
            nc.vector.tensor_tensor(out=atlo[:], in0=ps_at[:], in1=athi[:],
                                    op=SUB)
            nc.vector.tensor_copy(out=diag(atblk, 0), in_=athi[:])
            nc.vector.tensor_copy(out=diag(atblk, B), in_=atlo[:])

            # ---- ctx slice: attn @ enc[:, :, jsl] ----
            ps_cx = ps128.tile([2 * B, HSL], f32, name="ps_cx", tag="ps128")
            first = True
            for w in (0, 1):
                wid = 2 * B if w == 0 else B
                for b in range(B):
                    nc.tensor.matmul(
                        ps_cx[0:wid, :], atblk[:, 2 * B * b:2 * B * b + wid],
                        enctbj[w][:, b, :],
                        start=first, stop=(w == 1 and b == B - 1))
                    first = False
            cx_lo = work.tile([B, HSL], f32, tag="cx_lo", name="cx_lo")
            nc.scalar.activation(cx_lo[:], ps_cx[B:2 * B, :], AF.Copy)
            ctx_sl = work.tile([B, HSL], f32, tag="ctx_sl", name="ctx_sl")
            nc.vector.tensor_tensor(out=ctx_sl[:], in0=ps_cx[0:B, :],
                                    in1=cx_lo[:], op=ADD)
            ps_ct = ps128.tile([HSL, B], f32, name="ps_ct", tag="ps128")
            nc.tensor.transpose(ps_ct[:], ctx_sl[:], ident[0:B, 0:B])
            ctxT_sl = work.tile([HSL, B], f32, tag="ctxT_sl", name="ctxT_sl")
            nc.vector.tensor_copy(out=ctxT_sl[:], in_=ps_ct[:])

            # ---- AG2: ctx^T ----
            bounce2 = dram2.tile([pay], f32, name=f"bounce2_{t}", tag="bounce2")
            agout2 = dram2.tile([NCORES, pay], f32, addr_space="Shared",
                                name=f"agout2_{t}", tag="agout2")
            nc.sync.dma_start(
                bounce2[:].rearrange("(p f) -> p f", f=B), ctxT_sl[:])
            nc.gpsimd.collective_compute(
                "AllGather", mybir.AluOpType.bypass, replica_groups=rg,
                ins=[bounce2.opt()], outs=[agout2.opt()])
            ctxT = work.tile([P, KT, B], f32, tag="ctxT", name="ctxT")
            nc.sync.dma_start(
                ctxT[:], agout2[:].rearrange("r (p f) -> p r f", f=B))
            nc.scalar.activation(ctxTs[:, :, 0:B], ctxT[:], AF.Copy)
            tmpc = work.tile([P, KT, B], f32, tag="tmpc", name="tmpc")
            nc.vector.tensor_tensor(out=tmpc[:], in0=ctxT[:],
                                    in1=ctxTs[:, :, 0:B], op=SUB)
            nc.scalar.activation(ctxTs[:, :, B:2 * B], tmpc[:], AF.Copy)

            # ---- ctx part of gates (same psum group) ----
            for kt in range(KT):
                nc.tensor.matmul(ps_g[:], ctxTs[:, kt, :], wct_sb[0][:, kt, :],
                                 start=False, stop=False)
            for kt in range(KT):
                nc.tensor.matmul(ps_g[0:B, :], ctxTs[:, kt, 0:B],
                                 wct_sb[1][:, kt, :], start=False,
                                 stop=(kt == KT - 1))

            # ---- gates assembly + LSTM pointwise ----
            g_lo = work.tile([B, GSL], f32, tag="g_lo", name="g_lo")
            nc.scalar.activation(g_lo[:], ps_g[B:2 * B, :], AF.Copy)
            gsum = work.tile([B, GSL], f32, tag="gsum", name="gsum")
            nc.vector.tensor_tensor(out=gsum[:], in0=ps_g[0:B, :],
                                    in1=g_lo[:], op=ADD)
            xea_t = work.tile([B, GSL], f32, tag="xea_t", name="xea_t")
            nc.sync.dma_start(
                xea_t[:], xea_dram[B * (t % 4):B * (t % 4) + B, t // 4, :])
            gates = work.tile([B, GSL], f32, tag="gates", name="gates")
            nc.vector.tensor_tensor(out=gates[:], in0=gsum[:], in1=xea_t[:],
                                    op=ADD)
            sig_if = work.tile([B, 2 * HSL], f32, tag="sig_if", name="sig_if")
            nc.scalar.activation(sig_if[:], gates[:, 0:2 * HSL], AF.Sigmoid)
            tanh_g = work.tile([B, HSL], f32, tag="tanh_g", name="tanh_g")
            nc.scalar.activation(tanh_g[:], gates[:, 2 * HSL:3 * HSL], AF.Tanh)
            sig_o = work.tile([B, HSL], f32, tag="sig_o", name="sig_o")
            nc.scalar.activation(sig_o[:], gates[:, 3 * HSL:4 * HSL], AF.Sigmoid)
            tmp1 = work.tile([B, HSL], f32, tag="tmp1", name="tmp1")
            nc.vector.tensor_tensor(out=tmp1[:], in0=sig_if[:, HSL:2 * HSL],
                                    in1=c_st[:], op=MUL)
            tmp2 = work.tile([B, HSL], f32, tag="tmp2", name="tmp2")
            nc.vector.tensor_tensor(out=tmp2[:], in0=sig_if[:, 0:HSL],
                                    in1=tanh_g[:], op=MUL)
            nc.vector.tensor_tensor(out=c_st[:], in0=tmp1[:], in1=tmp2[:],
                                    op=ADD)
            tanh_c = work.tile([B, HSL], f32, tag="tanh_c", name="tanh_c")
            nc.scalar.activation(tanh_c[:], c_st[:], AF.Tanh)
            h2_sl = work.tile([B, HSL], f32, tag="h2_sl", name="h2_sl")
            nc.vector.tensor_tensor(out=h2_sl[:], in0=sig_o[:], in1=tanh_c[:],
                                    op=MUL)
            ps_h = ps128.tile([HSL, B], f32, name="ps_h", tag="ps128")
            nc.tensor.transpose(ps_h[:], h2_sl[:], ident[0:B, 0:B])
            nc.vector.tensor_copy(out=h2T_loc[:], in_=ps_h[:])

        # -------- P3: vocab projection (fp16, Wout streamed once) ----------
        for nt in range(VSL // NT):
            wo = wop.tile([P, KT, NT], f16, tag="wo", name="wo")
            nc.sync.dma_start(
                wo[:], d_woutt.ap().rearrange("(kt p) v -> p kt v", p=P)
                [:, :, nt * NT:(nt + 1) * NT])
            for mt in range(MT):
                pp = ps512.tile([P, NT], f32, name="pp", tag="ps512")
                for kt in range(KT):
                    nc.tensor.matmul(pp[:], h2tf[:, kt, mt * P:(mt + 1) * P],
                                     wo[:, kt, :],
                                     start=(kt == 0), stop=(kt == KT - 1))
                ot = otp.tile([P, NT], f32, tag="ot", name="ot")
                nc.vector.tensor_copy(out=ot[:], in_=pp[:])
                nc.sync.dma_start(
                    d_logits.ap()[mt * P:(mt + 1) * P, nt * NT:(nt + 1) * NT],
                    ot[:])

    nc.compile()
    return nc


def _split16(x):
    x = np.asarray(x, np.float32)
    h = x.astype(np.float16)
    l = (x - h.astype(np.float32)).astype(np.float16)
    return np.ascontiguousarray(h), np.ascontiguousarray(l)


def prep_inputs(input_var, add_var, h0, c0, enc_output, enc_mask, embed,
                Wa, Wih, Whh, bih, bhh, Wout, bout, T):
    """Host-side sharding / layout prep. Returns in_maps for the 8 cores."""
    f = np.float32
    input_var = np.asarray(input_var)
    tok_in = np.concatenate(
        [np.zeros((B, 1), input_var.dtype), input_var[:, :T - 1]], axis=1)
    embs = np.asarray(embed, f)[tok_in.astype(np.int64)]      # (B, T, E)
    X = np.concatenate([
        embs.transpose(1, 0, 2).reshape(T * B, E),
        np.tile(np.asarray(add_var, f), (T, 1))], axis=1)     # (T*B, 640)
    XeaInT = np.ascontiguousarray(X.T)
    WihT = np.asarray(Wih, f).T       # (1664, 4096)
    WhhT = np.asarray(Whh, f).T       # (1024, 4096)
    WaT = np.asarray(Wa, f).T         # (1024, 1024)
    WoutT = np.asarray(Wout, f).T     # (1024, 32000)
    bias = np.asarray(bih, f) + np.asarray(bhh, f)
    fmin = np.finfo(f).min
    maskb = np.where(np.asarray(enc_mask) > 0, f(0.0), fmin).astype(f)
    enc = np.asarray(enc_output, f)
    encTr = np.ascontiguousarray(
        enc.transpose(2, 0, 1).reshape(KT, P, B, TENC).transpose(1, 0, 2, 3))
    xeat_h, xeat_l = _split16(XeaInT)
    enctr_h, enctr_l = _split16(encTr)

    in_maps = []
    for c in range(NCORES):
        jsl = np.arange(c * HSL, (c + 1) * HSL)
        gcols = np.concatenate([jsl, H + jsl, 2 * H + jsl, 3 * H + jsl])
        vsl = slice(c * VSL, (c + 1) * VSL)
        weat_h, weat_l = _split16(WihT[0:E + A][:, gcols])
        wct_h, wct_l = _split16(WihT[E + A:][:, gcols])
        whht_h, whht_l = _split16(WhhT[:, gcols])
        watj_h, watj_l = _split16(WaT[:, jsl])
        etbj_h, etbj_l = _split16(
            enc.transpose(1, 0, 2)[:, :, jsl].reshape(TENC, B * HSL))
        in_maps.append({
            "xeat_h": xeat_h, "xeat_l": xeat_l,
            "weat_h": weat_h, "weat_l": weat_l,
            "wct_h": wct_h, "wct_l": wct_l,
            "whht_h": whht_h, "whht_l": whht_l,
            "watj_h": watj_h, "watj_l": watj_l,
            "enctr_h": enctr_h, "enctr_l": enctr_l,
            "enctbj_h": etbj_h, "enctbj_l": etbj_l,
            "woutt": np.ascontiguousarray(WoutT[:, vsl]).astype(np.float16),
            "h0tj": np.ascontiguousarray(np.asarray(h0, f)[:, jsl].T),
            "biasg": np.ascontiguousarray(bias[gcols])[None, :],
            "maskb": maskb,
            "c0j": np.ascontiguousarray(np.asarray(c0, f)[:, jsl]),
        })
    return in_maps


def run_decoder(inputs_dict, T, trace=False):
    if T not in _CACHE:
        _CACHE[T] = build_decoder(T)
    nc = _CACHE[T]
    in_maps = prep_inputs(T=T, **inputs_dict)
    res = bass_utils.run_bass_kernel_spmd(
        nc, in_maps, core_ids=list(range(NCORES)), trace=trace)
    out = np.empty((B, T, V), np.float32)
    for c in range(NCORES):
        out[:, :, c * VSL:(c + 1) * VSL] = (
            res.results[c]["logits"].reshape(T, B, VSL).transpose(1, 0, 2))
    out += np.asarray(inputs_dict["bout"], np.float32)[None, None, :]
    return out, res


def kernel(**inputs):
    T = np.asarray(inputs["input_var"]).shape[1]
    out, _ = run_decoder(inputs, T)
    return out



# revision 17
# speedup vs baseline: 97328.1592x; 97328.1592x over previous
"""Trainium2 Bass kernel for nn_Decoder (attention-LSTM decoder + vocab projection).

Sharding (8 NeuronCores, SPMD, rank-agnostic program; all rank dependence is
carried by per-core input data):
  - Hidden dim H=1024 (and matching i/f/g/o gate rows) sharded 8 ways.
  - Per decode step, ONE small AllGather carries [h2^T slice | partial
    attention scores]. The context->gates contribution is computed locally
    from the attention weights via encW[b, te, g] = sum_h enc[b,te,h] *
    Wih_ctx[h, gcols_j] (Wc folded into the encoder outputs, precomputed in
    P1), so no second collective is needed.
  - The output projection (dominant FLOPs) is vocab-sharded; the h2 history
    is accumulated in SBUF during the loop and projected in one pass at the
    end, streaming each core's Wout slice from HBM exactly once.

Precision: the LSTM recurrence amplifies per-step rounding noise by ~1000x
over 64 steps, so every matmul feeding the recurrence runs as an fp16 hi/lo
split (3 cross terms, fp32 PSUM accumulation => ~1e-6 error, measured on HW)
at full PE speed. lhsT hi/lo parts are stacked along M and rhs hi/lo parts
along N where possible so the cross terms share rhs streams. The final
vocab projection does not feed back, so single-pass f16 suffices.
Elementwise/state math is exact fp32.
"""

import sys

sys.path.insert(0, "/opt/trn_rl_repo")

import numpy as np

import concourse.mybir as mybir
import concourse.tile as tile
from concourse import bacc, bass_utils
from concourse.masks import make_identity

P = 128
B, TENC, V, E, H, A = 32, 128, 32000, 512, 1024, 128
NCORES = 8
HSL = H // NCORES          # 128 h-dims per core
GSL = 4 * HSL              # 512 gate rows per core
VSL = V // NCORES          # 4000 vocab per core
NT = 500                   # projection N chunk (4000 = 8 x 500)
KT = H // P                # 8 k-tiles over the hidden dim

f32 = mybir.dt.float32
f32r = mybir.dt.float32r
f16 = mybir.dt.float16
ADD = mybir.AluOpType.add
SUB = mybir.AluOpType.subtract
MUL = mybir.AluOpType.mult
AF = mybir.ActivationFunctionType

_CACHE = {}


def build_decoder(T, no_collective=False):
    # no_collective=True replaces the AllGather with a local DMA stand-in
    # (numerically WRONG; only for timing experiments via bench scripts).
    TB = T * B
    MT = TB // P
    nc = bacc.Bacc("TRN2", target_bir_lowering=False, debug=False,
                   num_devices=NCORES)

    def din(name, shape, dt_):
        return nc.dram_tensor(name, shape, dt_, kind="ExternalInput")

    # fp16 hi/lo pairs are prepared host-side for all static operands
    d_xeat = [din(f"xeat_{s}", [640, TB], f16) for s in "hl"]
    d_weat = [din(f"weat_{s}", [640, GSL], f16) for s in "hl"]
    d_wct = [din(f"wct_{s}", [H, GSL], f16) for s in "hl"]
    d_whht = [din(f"whht_{s}", [H, GSL], f16) for s in "hl"]
    d_watj = [din(f"watj_{s}", [H, HSL], f16) for s in "hl"]
    d_enctr = [din(f"enctr_{s}", [P, KT, B, TENC], f16) for s in "hl"]
    d_woutt = din("woutt", [H, VSL], f16)
    d_h0tj = din("h0tj", [HSL, B], f32)
    d_biasg = din("biasg", [1, GSL], f32)
    d_maskb = din("maskb", [B, TENC], f32)
    d_c0j = din("c0j", [B, HSL], f32)
    d_logits = nc.dram_tensor("logits", [TB, VSL], f32, kind="ExternalOutput")

    rg = [list(range(NCORES))]

    with tile.TileContext(nc) as tc:
      with tc.tile_pool(name="const", bufs=1) as const, \
           tc.tile_pool(name="dramc", bufs=1, space="DRAM") as dramc, \
           tc.tile_pool(name="dram2", bufs=2, space="DRAM") as dram2, \
           tc.tile_pool(name="ps512", bufs=3, space="PSUM") as ps512, \
           tc.tile_pool(name="ps128", bufs=5, space="PSUM") as ps128, \
           tc.tile_pool(name="work", bufs=2) as work, \
           tc.tile_pool(name="wop", bufs=2) as wop, \
           tc.tile_pool(name="otp", bufs=2) as otp:

        def ctile(shape, dt_, name):
            return const.tile(shape, dt_, name=name, tag=name)

        ident = ctile([P, P], f32, "ident")
        make_identity(nc, ident[:])
        maskb_sb = ctile([B, TENC], f32, "maskb_sb")
        nc.sync.dma_start(maskb_sb[:], d_maskb.ap())

        # ---- persistent P2 operands (fp16 hi/lo pairs) ----
        # encat: per-b Wa-folded enc^T, cols [hi TENC | lo TENC]
        encat = ctile([P, B, 2 * TENC], f16, "encat")
        # encw: per-b Wc-folded enc, [te, b, hi GSL | lo GSL]
        encw = ctile([P, B, 2 * GSL], f16, "encw")
        whht_sb = [ctile([P, KT, GSL], f16, f"whht_{s}") for s in "hl"]
        c_st = ctile([B, HSL], f32, "c_st")
        hT = ctile([P, KT, B], f32, "hT")
        hTs = ctile([P, KT, 2 * B], f16, "hTs")      # [hi | lo] stacked on M
        h2T_loc = ctile([HSL, B], f32, "h2T_loc")
        # per-b stacked block-diag lhsT tiles: cols [64b:64b+32] = hi diag,
        # [64b+32 : 64b+64] = lo diag (diag entry at col offset 65*b)
        scblk = ctile([P, 65 * B + B], f16, "scblk")
        atblk = ctile([P, 65 * B + B], f16, "atblk")
        nc.vector.memset(scblk[:], 0.0)
        nc.vector.memset(atblk[:], 0.0)
        # all steps of h^T, staged in DRAM (SBUF is too tight with encw)
        h2tf = dramc.tile([P, KT, TB], f16, name="h2tf", tag="h2tf")
        xea_dram = dramc.tile([P, MT, GSL], f32, name="xea_dram", tag="xea_dram")

        def diag(blk, off):
            # (128, 32) view with free stride 65: cols off + 65*b
            return blk[:, off:off + 65 * B].rearrange(
                "p (a c) -> p a c", c=65)[:, :, 0]

        for s in (0, 1):
            nc.sync.dma_start(
                whht_sb[s][:], d_whht[s].ap().rearrange("(kt p) g -> p kt g", p=P))
        nc.sync.dma_start(c_st[:], d_c0j.ap())
        nc.sync.dma_start(h2T_loc[:], d_h0tj.ap())

        # ---------------- P1a: Xea precompute ----------------
        with tc.tile_pool(name="p1", bufs=2) as p1, \
             tc.tile_pool(name="p1c", bufs=1) as p1c:
            onesf = p1c.tile([1, P], f32)
            nc.vector.memset(onesf[:], 1.0)
            biasg_sb = p1c.tile([1, GSL], f32)
            nc.sync.dma_start(biasg_sb[:], d_biasg.ap())
            biasb = p1c.tile([P, GSL], f32)
            pb = ps512.tile([P, GSL], f32, name="pb", tag="ps512")
            nc.tensor.matmul(pb[:], onesf[:], biasg_sb[:], start=True, stop=True)
            nc.vector.tensor_copy(out=biasb[:], in_=pb[:])
            weat_sb = [p1c.tile([P, 5, GSL], f16, name=f"weat{s}") for s in "hl"]
            for s in (0, 1):
                nc.sync.dma_start(
                    weat_sb[s][:],
                    d_weat[s].ap().rearrange("(kt p) g -> p kt g", p=P))
            # Xea[(t,b), g] = [emb|add] @ Wea + bias   (3-term fp16 split)
            for mt in range(MT):
                xin = [p1.tile([P, 5, P], f16, tag=f"xin{s}", name=f"xin{s}")
                       for s in "hl"]
                for s in (0, 1):
                    nc.sync.dma_start(
                        xin[s][:],
                        d_xeat[s].ap().rearrange("(kt p) m -> p kt m", p=P)
                        [:, :, mt * P:(mt + 1) * P])
                px = ps512.tile([P, GSL], f32, name="px", tag="ps512")
                first = True
                for (a, w) in ((0, 0), (0, 1), (1, 0)):
                    for kt in range(5):
                        nc.tensor.matmul(px[:], xin[a][:, kt, :],
                                         weat_sb[w][:, kt, :],
                                         start=first, stop=(a == 1 and kt == 4))
                        first = False
                xsb = p1.tile([P, GSL], f32, tag="xsb", name="xsb")
                nc.vector.tensor_tensor(out=xsb[:], in0=px[:],
                                        in1=biasb[:], op=ADD)
                nc.sync.dma_start(xea_dram[:, mt, :], xsb[:])

        # ---------------- P1b: EncA^T + encW precomputes ----------------
        with tc.tile_pool(name="p2b", bufs=2) as p1, \
             tc.tile_pool(name="p2bc", bufs=1) as p1c:
            watj_sb = [p1c.tile([P, KT, HSL], f16, name=f"watj{s}") for s in "hl"]
            wct_sb = [p1c.tile([P, KT, GSL], f16, name=f"wct{s}") for s in "hl"]
            for s in (0, 1):
                nc.sync.dma_start(
                    watj_sb[s][:],
                    d_watj[s].ap().rearrange("(kt p) j -> p kt j", p=P))
                nc.sync.dma_start(
                    wct_sb[s][:],
                    d_wct[s].ap().rearrange("(kt p) g -> p kt g", p=P))
            # Per b: EncA^T[j, b, te] = Wa[jsl, :] @ enc[b]^T and
            # encW[te, b, g] = enc[b] @ Wc[:, gcols]  (3-term, evict hi/lo)
            for b in range(B):
                etr = [p1.tile([P, KT, TENC], f16, tag=f"etr{s}",
                               name=f"etr{s}") for s in "hl"]
                for s in (0, 1):
                    nc.sync.dma_start(
                        etr[s][:], d_enctr[s].ap()[:, :, b, :])
                pa = ps512.tile([P, TENC], f32, name="pa", tag="ps512")
                first = True
                for (w, a) in ((0, 0), (0, 1), (1, 0)):
                    for kt in range(KT):
                        nc.tensor.matmul(
                            pa[:], watj_sb[w][:, kt, :], etr[a][:, kt, :],
                            start=first,
                            stop=(w == 1 and a == 0 and kt == KT - 1))
                        first = False
                tmpa = p1.tile([P, TENC], f32, tag="tmpa", name="tmpa")
                nc.scalar.activation(encat[:, b, 0:TENC], pa[:], AF.Copy)
                nc.vector.tensor_tensor(out=tmpa[:], in0=pa[:],
                                        in1=encat[:, b, 0:TENC], op=SUB)
                nc.scalar.activation(encat[:, b, TENC:2 * TENC], tmpa[:],
                                     AF.Copy)
                pw = ps512.tile([P, GSL], f32, name="pw", tag="ps512")
                first = True
                for (a, w) in ((0, 0), (1, 0), (0, 1)):
                    for kt in range(KT):
                        nc.tensor.matmul(
                            pw[:], etr[a][:, kt, :], wct_sb[w][:, kt, :],
                            start=first,
                            stop=(a == 0 and w == 1 and kt == KT - 1))
                        first = False
                tmpw = p1.tile([P, GSL], f32, tag="tmpw", name="tmpw")
                nc.scalar.activation(encw[:, b, 0:GSL], pw[:], AF.Copy)
                nc.vector.tensor_tensor(out=tmpw[:], in0=pw[:],
                                        in1=encw[:, b, 0:GSL], op=SUB)
                nc.scalar.activation(encw[:, b, GSL:2 * GSL], tmpw[:], AF.Copy)

        # ---------------- P2: recurrent loop ----------------
        for t in range(T + 1):
            last = t == T
            # ---- score partials from own h slice ----
            if not last:
                h2hi = work.tile([HSL, B], f16, tag="h2hi", name="h2hi")
                nc.scalar.activation(h2hi[:], h2T_loc[:], AF.Copy)
                h2lo = work.tile([HSL, B], f32, tag="h2lo", name="h2lo")
                nc.vector.tensor_tensor(out=h2lo[:], in0=h2T_loc[:],
                                        in1=h2hi[:], op=SUB)
                nc.vector.tensor_copy(out=diag(scblk, 0), in_=h2hi[:])
                nc.vector.tensor_copy(out=diag(scblk, B), in_=h2lo[:])
                # one matmul per b: rhs = [enc_hi | enc_lo] stacked on N;
                # rows 0:B = h_hi terms, rows B:2B = h_lo terms.
                ps_sc = ps128.tile([2 * B, 2 * TENC], f32, name="ps_sc",
                                   tag="ps128")
                for b in range(B):
                    nc.tensor.matmul(
                        ps_sc[:], scblk[:, 2 * B * b:2 * B * b + 2 * B],
                        encat[:, b, :],
                        start=(b == 0), stop=(b == B - 1))
                sc_t = work.tile([B, TENC], f32, tag="sc_t", name="sc_t")
                nc.scalar.activation(sc_t[:], ps_sc[0:B, TENC:2 * TENC],
                                     AF.Copy)
                sc_t2 = work.tile([B, TENC], f32, tag="sc_t2", name="sc_t2")
                nc.vector.tensor_tensor(out=sc_t2[:], in0=ps_sc[0:B, 0:TENC],
                                        in1=sc_t[:], op=ADD)
                sc_sb = work.tile([B, TENC], f32, tag="sc_sb", name="sc_sb")
                nc.vector.tensor_tensor(out=sc_sb[:],
                                        in0=ps_sc[B:2 * B, 0:TENC],
                                        in1=sc_t2[:], op=ADD)

            # ---- AG1: [h2T | score partial] ----
            pay = B * HSL
            bounce = dram2.tile([2 * pay], f32, name=f"bounce_{t}", tag="bounce")
            agout = dram2.tile([NCORES, 2 * pay], f32, addr_space="Shared",
                               name=f"agout_{t}", tag="agout")
            nc.sync.dma_start(
                bounce[0:pay].rearrange("(p f) -> p f", f=B), h2T_loc[:])
            if not last:
                nc.sync.dma_start(
                    bounce[pay:2 * pay].rearrange("(c f) -> c f", f=TENC),
                    sc_sb[:])
            if no_collective:
                nc.sync.dma_start(
                    agout[0:1, :], bounce[:].rearrange("(r f) -> r f", r=1))
            else:
                nc.gpsimd.collective_compute(
                    "AllGather", mybir.AluOpType.bypass, replica_groups=rg,
                    ins=[bounce.opt()], outs=[agout.opt()])
            nc.sync.dma_start(
                hT[:], agout[:, 0:pay].rearrange("r (p f) -> p r f", f=B))

            # stash h^T (h2 of step t-1) for the end-of-loop projection
            if t >= 1:
                h2s = work.tile([P, KT, B], f16, tag="h2s", name="h2s")
                nc.scalar.activation(h2s[:], hT[:], AF.Copy)
                nc.sync.dma_start(h2tf[:, :, B * (t - 1):B * t], h2s[:])
            if last:
                break

            # hi/lo stack of full h^T (for the Whh matmul)
            nc.scalar.activation(hTs[:, :, 0:B], hT[:], AF.Copy)
            tmph = work.tile([P, KT, B], f32, tag="tmph", name="tmph")
            nc.vector.tensor_tensor(out=tmph[:], in0=hT[:],
                                    in1=hTs[:, :, 0:B], op=SUB)
            nc.scalar.activation(hTs[:, :, B:2 * B], tmph[:], AF.Copy)

            # gates psum: h part first (independent of softmax)
            ps_g = ps512.tile([2 * B, GSL], f32, name="ps_g", tag="ps512")
            for kt in range(KT):
                nc.tensor.matmul(ps_g[:], hTs[:, kt, :], whht_sb[0][:, kt, :],
                                 start=(kt == 0), stop=False)
            for kt in range(KT):
                nc.tensor.matmul(ps_g[0:B, :], hTs[:, kt, 0:B],
                                 whht_sb[1][:, kt, :], start=False, stop=False)

            # ---- scores -> softmax ----
            sc8 = work.tile([B, NCORES, TENC], f32, tag="sc8", name="sc8", bufs=1)
            nc.sync.dma_start(
                sc8[:],
                agout[:, pay:2 * pay].rearrange("r (c f) -> c r f", f=TENC))
            scores = work.tile([B, TENC], f32, tag="scores", name="scores")
            nc.vector.reduce_sum(scores[:], sc8[:].rearrange("c r f -> c f r"),
                                 axis=mybir.AxisListType.X)
            nc.vector.tensor_tensor(out=scores[:], in0=scores[:],
                                    in1=maskb_sb[:], op=ADD)
            negmax = work.tile([B, 1], f32, tag="negmax", name="negmax")
            nc.vector.reduce_max(negmax[:], scores[:],
                                 axis=mybir.AxisListType.X, negate=True)
            attn_e = work.tile([B, TENC], f32, tag="attn_e", name="attn_e")
            sumexp = work.tile([B, 1], f32, tag="sumexp", name="sumexp")
            nc.scalar.activation(attn_e[:], scores[:], AF.Exp,
                                 bias=negmax[:], scale=1.0, accum_out=sumexp[:])
            recip = work.tile([B, 1], f32, tag="recip", name="recip")
            nc.vector.reciprocal(recip[:], sumexp[:])
            attn_n = work.tile([B, TENC], f32, tag="attn_n", name="attn_n")
            nc.vector.tensor_scalar_mul(attn_n[:], attn_e[:], recip[:])

            # attn^T hi/lo into block-diag
            ps_at = ps128.tile([TENC, B], f32, name="ps_at", tag="ps128")
            nc.tensor.transpose(ps_at[:], attn_n[:], ident[0:B, 0:B])
            athi = work.tile([TENC, B], f16, tag="athi", name="athi")
            nc.scalar.activation(athi[:], ps_at[:], AF.Copy)
            atlo = work.tile([TENC, B], f32, tag="atlo", name="atlo")
            nc.vector.tensor_tensor(out=atlo[:], in0=ps_at[:], in1=athi[:],
                                    op=SUB)
            nc.vector.tensor_copy(out=diag(atblk, 0), in_=athi[:])
            nc.vector.tensor_copy(out=diag(atblk, B), in_=atlo[:])

            # ---- ctx part of gates via encW (same psum group, no AG2) ----
            # per b: rows b += at_hi . encw_hi[b], rows B+b += at_lo . encw_hi
            # then   rows b += at_hi . encw_lo[b]
            for b in range(B):
                nc.tensor.matmul(
                    ps_g[:], atblk[:, 2 * B * b:2 * B * b + 2 * B],
                    encw[:, b, 0:GSL], start=False, stop=False)
            for b in range(B):
                nc.tensor.matmul(
                    ps_g[0:B, :], atblk[:, 2 * B * b:2 * B * b + B],
                    encw[:, b, GSL:2 * GSL], start=False,
                    stop=(b == B - 1))

            # ---- gates assembly + LSTM pointwise ----
            g_lo = work.tile([B, GSL], f32, tag="g_lo", name="g_lo")
            nc.scalar.activation(g_lo[:], ps_g[B:2 * B, :], AF.Copy)
            gsum = work.tile([B, GSL], f32, tag="gsum", name="gsum")
            nc.vector.tensor_tensor(out=gsum[:], in0=ps_g[0:B, :],
                                    in1=g_lo[:], op=ADD)
            xea_t = work.tile([B, GSL], f32, tag="xea_t", name="xea_t")
            nc.sync.dma_start(
                xea_t[:], xea_dram[B * (t % 4):B * (t % 4) + B, t // 4, :])
            gates = work.tile([B, GSL], f32, tag="gates", name="gates")
            nc.vector.tensor_tensor(out=gates[:], in0=gsum[:], in1=xea_t[:],
                                    op=ADD)
            sig_if = work.tile([B, 2 * HSL], f32, tag="sig_if", name="sig_if")
            nc.scalar.activation(sig_if[:], gates[:, 0:2 * HSL], AF.Sigmoid)
            tanh_g = work.tile([B, HSL], f32, tag="tanh_g", name="tanh_g")
            nc.scalar.activation(tanh_g[:], gates[:, 2 * HSL:3 * HSL], AF.Tanh)
            sig_o = work.tile([B, HSL], f32, tag="sig_o", name="sig_o")
            nc.scalar.activation(sig_o[:], gates[:, 3 * HSL:4 * HSL], AF.Sigmoid)
            tmp1 = work.tile([B, HSL], f32, tag="tmp1", name="tmp1")
            nc.vector.tensor_tensor(out=tmp1[:], in0=sig_if[:, HSL:2 * HSL],
                                    in1=c_st[:], op=MUL)
            tmp2 = work.tile([B, HSL], f32, tag="tmp2", name="tmp2")
            nc.vector.tensor_tensor(out=tmp2[:], in0=sig_if[:, 0:HSL],
                                    in1=tanh_g[:], op=MUL)
            nc.vector.tensor_tensor(out=c_st[:], in0=tmp1[:], in1=tmp2[:],
                                    op=ADD)
            tanh_c = work.tile([B, HSL], f32, tag="tanh_c", name="tanh_c")
            nc.scalar.activation(tanh_c[:], c_st[:], AF.Tanh)
            h2_sl = work.tile([B, HSL], f32, tag="h2_sl", name="h2_sl")
            nc.vector.tensor_tensor(out=h2_sl[:], in0=sig_o[:], in1=tanh_c[:],
                                    op=MUL)
            ps_h = ps128.tile([HSL, B], f32, name="ps_h", tag="ps128")
            nc.tensor.transpose(ps_h[:], h2_sl[:], ident[0:B, 0:B])
            nc.vector.tensor_copy(out=h2T_loc[:], in_=ps_h[:])

        # -------- P3: vocab projection (fp16, Wout streamed once) ----------
        with tc.tile_pool(name="h2p", bufs=2) as h2p:
            for nt in range(VSL // NT):
                wo = wop.tile([P, KT, NT], f16, tag="wo", name="wo")
                nc.sync.dma_start(
                    wo[:], d_woutt.ap().rearrange("(kt p) v -> p kt v", p=P)
                    [:, :, nt * NT:(nt + 1) * NT])
                for mt in range(MT):
                    h2c = h2p.tile([P, KT, P], f16, tag="h2c", name="h2c")
                    nc.sync.dma_start(
                        h2c[:], h2tf[:, :, mt * P:(mt + 1) * P])
                    pp = ps512.tile([P, NT], f32, name="pp", tag="ps512")
                    for kt in range(KT):
                        nc.tensor.matmul(pp[:], h2c[:, kt, :],
                                         wo[:, kt, :],
                                         start=(kt == 0), stop=(kt == KT - 1))
                    ot = otp.tile([P, NT], f32, tag="ot", name="ot")
                    nc.vector.tensor_copy(out=ot[:], in_=pp[:])
                    nc.sync.dma_start(
                        d_logits.ap()[mt * P:(mt + 1) * P,
                                      nt * NT:(nt + 1) * NT],
                        ot[:])

    nc.compile()
    return nc


def _split16(x):
    x = np.asarray(x, np.float32)
    h = x.astype(np.float16)
    l = (x - h.astype(np.float32)).astype(np.float16)
    return np.ascontiguousarray(h), np.ascontiguousarray(l)


def prep_inputs(input_var, add_var, h0, c0, enc_output, enc_mask, embed,
                Wa, Wih, Whh, bih, bhh, Wout, bout, T):
    """Host-side sharding / layout prep. Returns in_maps for the 8 cores."""
    f = np.float32
    input_var = np.asarray(input_var)
    tok_in = np.concatenate(
        [np.zeros((B, 1), input_var.dtype), input_var[:, :T - 1]], axis=1)
    embs = np.asarray(embed, f)[tok_in.astype(np.int64)]      # (B, T, E)
    X = np.concatenate([
        embs.transpose(1, 0, 2).reshape(T * B, E),
        np.tile(np.asarray(add_var, f), (T, 1))], axis=1)     # (T*B, 640)
    XeaInT = np.ascontiguousarray(X.T)
    WihT = np.asarray(Wih, f).T       # (1664, 4096)
    WhhT = np.asarray(Whh, f).T       # (1024, 4096)
    WaT = np.asarray(Wa, f).T         # (1024, 1024)
    WoutT = np.asarray(Wout, f).T     # (1024, 32000)
    bias = np.asarray(bih, f) + np.asarray(bhh, f)
    fmin = np.finfo(f).min
    maskb = np.where(np.asarray(enc_mask) > 0, f(0.0), fmin).astype(f)
    enc = np.asarray(enc_output, f)
    encTr = np.ascontiguousarray(
        enc.transpose(2, 0, 1).reshape(KT, P, B, TENC).transpose(1, 0, 2, 3))
    xeat_h, xeat_l = _split16(XeaInT)
    enctr_h, enctr_l = _split16(encTr)

    in_maps = []
    for c in range(NCORES):
        jsl = np.arange(c * HSL, (c + 1) * HSL)
        gcols = np.concatenate([jsl, H + jsl, 2 * H + jsl, 3 * H + jsl])
        vsl = slice(c * VSL, (c + 1) * VSL)
        weat_h, weat_l = _split16(WihT[0:E + A][:, gcols])
        wct_h, wct_l = _split16(WihT[E + A:][:, gcols])
        whht_h, whht_l = _split16(WhhT[:, gcols])
        watj_h, watj_l = _split16(WaT[:, jsl])
        in_maps.append({
            "xeat_h": xeat_h, "xeat_l": xeat_l,
            "weat_h": weat_h, "weat_l": weat_l,
            "wct_h": wct_h, "wct_l": wct_l,
            "whht_h": whht_h, "whht_l": whht_l,
            "watj_h": watj_h, "watj_l": watj_l,
            "enctr_h": enctr_h, "enctr_l": enctr_l,
            "woutt": np.ascontiguousarray(WoutT[:, vsl]).astype(np.float16),
            "h0tj": np.ascontiguousarray(np.asarray(h0, f)[:, jsl].T),
            "biasg": np.ascontiguousarray(bias[gcols])[None, :],
            "maskb": maskb,
            "c0j": np.ascontiguousarray(np.asarray(c0, f)[:, jsl]),
        })
    return in_maps


def run_decoder(inputs_dict, T, trace=False):
    if T not in _CACHE:
        _CACHE[T] = build_decoder(T)
    nc = _CACHE[T]
    in_maps = prep_inputs(T=T, **inputs_dict)
    res = bass_utils.run_bass_kernel_spmd(
        nc, in_maps, core_ids=list(range(NCORES)), trace=trace)
    out = np.empty((B, T, V), np.float32)
    for c in range(NCORES):
        out[:, :, c * VSL:(c + 1) * VSL] = (
            res.results[c]["logits"].reshape(T, B, VSL).transpose(1, 0, 2))
    out += np.asarray(inputs_dict["bout"], np.float32)[None, None, :]
    return out, res


def kernel(**inputs):
    T = np.asarray(inputs["input_var"]).shape[1]
    out, _ = run_decoder(inputs, T)
    return out

